# revision 15
# baseline (speedup 1.0000x reference)
"""BitLinear 1.58 (nn_BitLinear158) Trainium2 Bass kernel.

Problem: x:[4,2048,4096] f32, weight:[4096,4096] f32 ->
         absmax-group-quantized x (8-bit fake quant, groups of 64) @
         ternary-quantized weight.T (per-row absmean scale) -> [4,2048,4096].

Sharding: data-parallel over tokens (1024 tokens/core, full weight
replicated) — minimizes replicated elementwise work.

Per-core kernel (M=1024, K=4096, O=4096), engine-balanced so the tensor
engine (437us roofline at 2.4GHz) paces the pipeline:
  - PE: stationary = ternary weights [k,128o], moving = x_q [k,512m],
    psum [o,m]; 2048 matmuls stream at ~216ns each (full clock,
    ldweights pipelined).
  - DVE: the group reduces (x absmax, w abs-sum; reduces are DVE-only),
    the small scale chain, and the x round. ~300us.
  - Act (scalar): w loads (HWDGE), the two Sign passes per w half-tile
    (ternarize via sign(w-s/2)+sign(w+s/2), boundary fixed by a
    grid-of-2 magic round), psum eviction via Copy activation with
    scale=0.5*s as a per-partition AP (output orientation [o,m] makes
    the row scale per-partition). ~230us.
  - Pool (gpsimd): x scale-mult + dequant passes, w sign-sum combine +
    magic fix (USE_POOL), plus the SWDGE output stores. ~270us.
  - SP (sync): x loads + all xbar transposes (keeping transpose issue
    off the Act queue, which would head-of-line block behind evictions).
  - s computed with a two-stage compensated reduction (exact 2^-12-grid
    split) to track the f32 reference mean closely; ternary decisions
    are exact comparisons against +-0.5*s so there is no divide rounding.
"""
import sys

sys.path.insert(0, "/opt/trn_rl_repo")

import numpy as np

B, S, D_IN, D_OUT = 4, 2048, 4096, 4096
N_CORES = 8
M_TOT = B * S
M_C = M_TOT // N_CORES

P = 128
G = 64
MAGIC = float(1.5 * 2.0 ** 23)   # fp32 round-to-nearest-even trick
MAGIC2 = float(1.5 * 2.0 ** 11)  # quantize-to-2^-12-grid trick
FIXC = float(1.5 * 2.0 ** 24)    # f32 grid-of-2 round (ties-to-even)
EPS = 1e-5
QMAX = 127.0
INV_QMAX = float(np.float32(1.0 / 127.0))

# Note: gpsimd/Pool compute was measured and rejected — Pool TENSOR_SCALAR
# runs ~29us/[128,2048] (software DSP path) and even Pool TENSOR_TENSOR
# (~3.6us) degrades concurrent DVE throughput ~2x via SBUF port contention.

_cache = {}


def _build(M, K, O):
    import concourse.bass as bass
    import concourse.tile as tile
    from concourse import bacc, mybir

    f32 = mybir.dt.float32
    f16 = mybir.dt.float16
    Alu = mybir.AluOpType
    Act = mybir.ActivationFunctionType
    Ax = mybir.AxisListType

    K2 = K // 2          # 2048, half-row staged per DMA
    KSUB = K // P        # 32 contraction chunks
    KS2 = KSUB // 2      # 16 chunks per half
    MB = M // P          # token blocks
    OC = O // P          # out-feature blocks
    MH = M // 512        # psum column halves
    NGX = K2 // G        # 32 quant groups per x half-tile
    NGW = K // G         # 64 abs-mean groups per w row

    nc = bacc.Bacc("TRN2", target_bir_lowering=False, num_devices=1)
    x = nc.dram_tensor("x", [M, K], f32, kind="ExternalInput")
    w = nc.dram_tensor("w", [O, K], f32, kind="ExternalInput")
    # transposed output [O, M]; host untransposes at gather time
    out = nc.dram_tensor("out", [O, M], f32, kind="ExternalOutput")

    xap, wap, oap = x.ap(), w.ap(), out.ap()

    with tile.TileContext(nc) as tc:
        with (
            tc.tile_pool(name="xq", bufs=1) as xq_pool,
            tc.tile_pool(name="xst", bufs=4) as xst,
            tc.tile_pool(name="xq16", bufs=2) as xq16_pool,
            tc.tile_pool(name="wst", bufs=4) as wst,
            tc.tile_pool(name="sg", bufs=4) as sg_pool,
            tc.tile_pool(name="wt", bufs=3) as wt_pool,
            tc.tile_pool(name="small", bufs=3) as small,
            tc.tile_pool(name="sv", bufs=8) as sv_pool,
            tc.tile_pool(name="ev", bufs=2) as ev_pool,
            tc.tile_pool(name="ps", bufs=6, space="PSUM") as ps_pool,
        ):
            # -------- activation loads (SP queue, issued first) --------
            x_stage = {}
            for mb in range(MB):
                for h in range(2):
                    xt = xst.tile([P, K2], f32, tag="xst",
                                  name=f"xt{mb}_{h}")
                    nc.sync.dma_start(
                        xt[:], xap[mb * P:(mb + 1) * P, h * K2:(h + 1) * K2])
                    x_stage[(mb, h)] = xt

            # -------- weight loads (SP queue, lookahead) --------
            w_stage = {}

            def wload(oc):
                tiles = []
                for h in range(2):
                    wh = wst.tile([P, K2], f32, tag="wst",
                                  name=f"wh{oc}_{h}")
                    nc.sync.dma_start(
                        wh[:], wap[oc * P:(oc + 1) * P, h * K2:(h + 1) * K2])
                    tiles.append(wh)
                w_stage[oc] = tiles

            for _oc in range(min(2, OC)):
                wload(_oc)

            # -------- activation quantization + transpose --------
            xq_t = xq_pool.tile([P, KSUB, M], f16, name="xq_t")
            for mb in range(MB):
                for h in range(2):
                    xt = x_stage.pop((mb, h))
                    xg = xt.rearrange("p (g e) -> p g e", e=G)
                    am = small.tile([P, NGX], f32, tag="am")
                    nc.vector.tensor_reduce(am[:], xg, Ax.X, Alu.max,
                                            apply_absolute_value=True)
                    am2 = small.tile([P, NGX], f32, tag="am2")
                    nc.vector.tensor_scalar(am2[:], am[:], EPS, None, Alu.max)
                    rc = small.tile([P, NGX], f32, tag="rc")
                    nc.vector.reciprocal(rc[:], am2[:])
                    scale = small.tile([P, NGX], f32, tag="scale")
                    nc.vector.tensor_scalar(scale[:], rc[:], QMAX, None,
                                            Alu.mult)
                    inv = small.tile([P, NGX], f32, tag="inv")
                    nc.vector.tensor_scalar(inv[:], am2[:], INV_QMAX, None,
                                            Alu.mult)
                    # xs = x * scale (group-broadcast)
                    nc.vector.tensor_tensor(
                        xg, xg,
                        scale[:, :, None].to_broadcast((P, NGX, G)),
                        Alu.mult)
                    # q = rint(xs) via magic add/sub
                    nc.vector.tensor_scalar(xt[:], xt[:], MAGIC, MAGIC,
                                            Alu.add, Alu.subtract)
                    # x_q = q * (absmax/127) -> fp16
                    xq16 = xq16_pool.tile([P, K2], f16, tag="xq16",
                                          name=f"xq16_{mb}_{h}")
                    nc.vector.tensor_tensor(
                        xq16.rearrange("p (g e) -> p g e", e=G), xg,
                        inv[:, :, None].to_broadcast((P, NGX, G)),
                        Alu.mult)
                    nc.sync.dma_start_transpose(
                        xq_t[:, h * KS2:(h + 1) * KS2,
                             mb * P:(mb + 1) * P], xq16[:])

            # -------- weight ternarize --------
            wt_tiles = {}
            bp_tiles = {}

            def wternarize(oc):
                whs = w_stage.pop(oc)
                gs = small.tile([P, NGW], f32, tag="gs")
                for h in range(2):
                    nc.vector.tensor_reduce(
                        gs[:, h * NGX:(h + 1) * NGX],
                        whs[h].rearrange("p (g e) -> p g e", e=G),
                        Ax.X, Alu.add, apply_absolute_value=True)
                # s = max(mean|row|, eps), two-stage compensated sum
                hq = small.tile([P, NGW], f32, tag="hq")
                nc.vector.tensor_scalar(hq[:], gs[:], MAGIC2, MAGIC2,
                                        Alu.add, Alu.subtract)
                lq = small.tile([P, NGW], f32, tag="lq")
                nc.vector.tensor_tensor(lq[:], gs[:], hq[:], Alu.subtract)
                sh = small.tile([P, 1], f32, tag="sh")
                nc.vector.tensor_reduce(sh[:], hq[:], Ax.X, Alu.add)
                sl = small.tile([P, 1], f32, tag="sl")
                nc.vector.tensor_reduce(sl[:], lq[:], Ax.X, Alu.add)
                ssum = small.tile([P, 1], f32, tag="ssum")
                nc.vector.tensor_tensor(ssum[:], sh[:], sl[:], Alu.add)
                sv = small.tile([P, 1], f32, tag="svv")
                nc.vector.tensor_scalar(sv[:], ssum[:],
                                        float(np.float32(1.0 / K)),
                                        EPS, Alu.mult, Alu.max)
                # eviction scale is 0.5*s (the sign-sum below is 2t)
                bp = sv_pool.tile([P, 1], f32, tag="bp", name=f"bp{oc}")
                nc.vector.tensor_scalar(bp[:], sv[:], 0.5, None, Alu.mult)
                bp_tiles[oc] = bp
                bn = small.tile([P, 1], f32, tag="bn")
                nc.vector.tensor_scalar(bn[:], sv[:], -0.5, None, Alu.mult)
                # 2t = sign(w-0.5s) + sign(w+0.5s); exact comparisons.
                # Boundary |w|==0.5s gives +-1; the f32 grid-of-2 magic
                # round maps it to 0 (= round-half-even of w/s).
                wt = wt_pool.tile([P, KSUB, P], f16, tag="wt",
                                  name=f"wt{oc}")
                wt_tiles[oc] = wt
                for h in range(2):
                    sga = sg_pool.tile([P, K2], f16, tag="sga",
                                       name=f"sga{oc}_{h}")
                    nc.scalar.activation(out=sga[:], in_=whs[h][:],
                                         func=Act.Sign, bias=bn[:],
                                         scale=1.0)
                    sgb = sg_pool.tile([P, K2], f16, tag="sgb",
                                       name=f"sgb{oc}_{h}")
                    nc.scalar.activation(out=sgb[:], in_=whs[h][:],
                                         func=Act.Sign, bias=bp[:],
                                         scale=1.0)
                    nc.vector.tensor_tensor(sga[:], sga[:], sgb[:], Alu.add)
                    nc.vector.tensor_scalar(sga[:], sga[:], FIXC, FIXC,
                                            Alu.add, Alu.subtract)
                    nc.sync.dma_start_transpose(
                        wt[:, h * KS2:(h + 1) * KS2, :], sga[:])

            # -------- matmul + eviction --------
            # Ternarize runs one oc ahead of the matmuls and evictions run
            # one oc behind, so the Act queue never head-of-line blocks the
            # next oc's Sign passes behind an eviction that waits on PE.
            ps_tiles = {}

            def evict(oc):
                bp = bp_tiles.pop(oc)
                for mh in range(MH):
                    ps = ps_tiles.pop((oc, mh))
                    ev = ev_pool.tile([P, 512], f32)
                    nc.scalar.activation(out=ev[:], in_=ps[:],
                                         func=Act.Copy, scale=bp[:])
                    nc.gpsimd.dma_start(
                        oap[oc * P:(oc + 1) * P, mh * 512:(mh + 1) * 512],
                        ev[:])

            wternarize(0)
            for oc in range(OC):
                if oc + 2 < OC:
                    wload(oc + 2)
                if oc + 1 < OC:
                    wternarize(oc + 1)
                wt = wt_tiles.pop(oc)
                for mh in range(MH):
                    ps = ps_pool.tile([P, 512], f32)
                    ps_tiles[(oc, mh)] = ps
                    for ks in range(KSUB):
                        nc.tensor.matmul(
                            ps[:], wt[:, ks, :],
                            xq_t[:, ks, mh * 512:(mh + 1) * 512],
                            start=(ks == 0), stop=(ks == KSUB - 1))
                if oc >= 1:
                    evict(oc - 1)
            evict(OC - 1)

    nc.compile()
    return nc


def _get_nc():
    if "nc" not in _cache:
        _cache["nc"] = _build(M_C, D_IN, D_OUT)
    return _cache["nc"]


def run(x, weight, trace=False):
    """Run on 8 NeuronCores; returns (full output [B,S,D_OUT], results obj)."""
    from concourse.bass_utils import run_bass_kernel_spmd

    x = np.ascontiguousarray(np.asarray(x, dtype=np.float32))
    w = np.ascontiguousarray(np.asarray(weight, dtype=np.float32))
    assert x.shape == (B, S, D_IN) and w.shape == (D_OUT, D_IN)
    xf = x.reshape(M_TOT, D_IN)
    nc = _get_nc()
    in_maps = [
        {"x": np.ascontiguousarray(xf[c * M_C:(c + 1) * M_C]), "w": w}
        for c in range(N_CORES)
    ]
    res = run_bass_kernel_spmd(nc, in_maps, core_ids=list(range(N_CORES)),
                               trace=trace)
    outf = np.concatenate(
        [res.results[c]["out"].T for c in range(N_CORES)], axis=0)
    return np.ascontiguousarray(outf).reshape(B, S, D_OUT), res


def kernel(x, weight):
    out, _ = run(x, weight)
    return out


# revision 19
# speedup vs baseline: 1.0341x; 1.0341x over previous
"""BitLinear 1.58 (nn_BitLinear158) Trainium2 Bass kernel.

Problem: x:[4,2048,4096] f32, weight:[4096,4096] f32 ->
         absmax-group-quantized x (8-bit fake quant, groups of 64) @
         ternary-quantized weight.T (per-row absmean scale) -> [4,2048,4096].

Sharding: data-parallel over tokens (1024 tokens/core, full weight
replicated) — minimizes replicated elementwise work.

Per-core kernel (M=1024, K=4096, O=4096), engine-balanced so the tensor
engine (437us roofline at 2.4GHz) paces the pipeline:
  - PE: stationary = ternary weights [k,128o], moving = x_q [k,512m],
    psum [o,m]; 2048 matmuls stream at ~216ns each (full clock,
    ldweights pipelined).
  - DVE: the group reduces (x absmax, w abs-sum; reduces are DVE-only),
    the small scale chain, and the x round. ~300us.
  - Act (scalar): w loads (HWDGE), the two Sign passes per w half-tile
    (ternarize via sign(w-s/2)+sign(w+s/2), boundary fixed by a
    grid-of-2 magic round), psum eviction via Copy activation with
    scale=0.5*s as a per-partition AP (output orientation [o,m] makes
    the row scale per-partition). ~230us.
  - Pool (gpsimd): x scale-mult + dequant passes, w sign-sum combine +
    magic fix (USE_POOL), plus the SWDGE output stores. ~270us.
  - SP (sync): x loads + all xbar transposes (keeping transpose issue
    off the Act queue, which would head-of-line block behind evictions).
  - s computed with a two-stage compensated reduction (exact 2^-12-grid
    split) to track the f32 reference mean closely; ternary decisions
    are exact comparisons against +-0.5*s so there is no divide rounding.
"""
import sys

sys.path.insert(0, "/opt/trn_rl_repo")

import numpy as np

B, S, D_IN, D_OUT = 4, 2048, 4096, 4096
N_CORES = 8
M_TOT = B * S
M_C = M_TOT // N_CORES

P = 128
G = 64
MAGIC = float(1.5 * 2.0 ** 23)   # fp32 round-to-nearest-even trick
MAGIC2 = float(1.5 * 2.0 ** 11)  # quantize-to-2^-12-grid trick
FIXC = float(1.5 * 2.0 ** 24)    # f32 grid-of-2 round (ties-to-even)
EPS = 1e-5
QMAX = 127.0
INV_QMAX = float(np.float32(1.0 / 127.0))

# Note: gpsimd/Pool compute was measured and rejected — Pool TENSOR_SCALAR
# runs ~29us/[128,2048] (software DSP path) and even Pool TENSOR_TENSOR
# (~3.6us) degrades concurrent DVE throughput ~2x via SBUF port contention.

_cache = {}


def _build(M, K, O):
    import concourse.bass as bass
    import concourse.tile as tile
    from concourse import bacc, mybir

    f32 = mybir.dt.float32
    f16 = mybir.dt.float16
    Alu = mybir.AluOpType
    Act = mybir.ActivationFunctionType
    Ax = mybir.AxisListType

    K2 = K // 2          # 2048, half-row staged per DMA
    KSUB = K // P        # 32 contraction chunks
    KS2 = KSUB // 2      # 16 chunks per half
    MB = M // P          # token blocks
    OC = O // P          # out-feature blocks
    MH = M // 512        # psum column halves
    NGX = K2 // G        # 32 quant groups per x half-tile
    NGW = K // G         # 64 abs-mean groups per w row

    nc = bacc.Bacc("TRN2", target_bir_lowering=False, num_devices=1)
    x = nc.dram_tensor("x", [M, K], f32, kind="ExternalInput")
    w = nc.dram_tensor("w", [O, K], f32, kind="ExternalInput")
    # transposed output [O, M]; host untransposes at gather time
    out = nc.dram_tensor("out", [O, M], f32, kind="ExternalOutput")

    xap, wap, oap = x.ap(), w.ap(), out.ap()

    with tile.TileContext(nc) as tc:
        with (
            tc.tile_pool(name="xq", bufs=1) as xq_pool,
            tc.tile_pool(name="xst", bufs=3) as xst,
            tc.tile_pool(name="xq16", bufs=2) as xq16_pool,
            tc.tile_pool(name="wst", bufs=6) as wst,
            tc.tile_pool(name="sg", bufs=3) as sg_pool,
            tc.tile_pool(name="wt", bufs=3) as wt_pool,
            tc.tile_pool(name="small", bufs=2) as small,
            tc.tile_pool(name="sv", bufs=8) as sv_pool,
            tc.tile_pool(name="ev", bufs=2) as ev_pool,
            tc.tile_pool(name="ps", bufs=6, space="PSUM") as ps_pool,
        ):
            # -------- activation loads (SP queue, issued first) --------
            x_stage = {}
            for mb in range(MB):
                for h in range(2):
                    xt = xst.tile([P, K2], f32, tag="xst",
                                  name=f"xt{mb}_{h}")
                    nc.sync.dma_start(
                        xt[:], xap[mb * P:(mb + 1) * P, h * K2:(h + 1) * K2])
                    x_stage[(mb, h)] = xt

            # -------- weight loads (SP queue, lookahead) --------
            w_stage = {}

            def wload(oc):
                tiles = []
                for h in range(2):
                    wh = wst.tile([P, K2], f32, tag="wst",
                                  name=f"wh{oc}_{h}")
                    nc.sync.dma_start(
                        wh[:], wap[oc * P:(oc + 1) * P, h * K2:(h + 1) * K2])
                    tiles.append(wh)
                w_stage[oc] = tiles

            for _oc in range(min(3, OC)):
                wload(_oc)

            # -------- activation quantization + transpose --------
            xq_t = xq_pool.tile([P, KSUB, M], f16, name="xq_t")
            for mb in range(MB):
                for h in range(2):
                    xt = x_stage.pop((mb, h))
                    xg = xt.rearrange("p (g e) -> p g e", e=G)
                    am = small.tile([P, NGX], f32, tag="am")
                    nc.vector.tensor_reduce(am[:], xg, Ax.X, Alu.max,
                                            apply_absolute_value=True)
                    am2 = small.tile([P, NGX], f32, tag="am2")
                    nc.vector.tensor_scalar(am2[:], am[:], EPS, None, Alu.max)
                    rc = small.tile([P, NGX], f32, tag="rc")
                    nc.vector.reciprocal(rc[:], am2[:])
                    scale = small.tile([P, NGX], f32, tag="scale")
                    nc.vector.tensor_scalar(scale[:], rc[:], QMAX, None,
                                            Alu.mult)
                    inv = small.tile([P, NGX], f32, tag="inv")
                    nc.vector.tensor_scalar(inv[:], am2[:], INV_QMAX, None,
                                            Alu.mult)
                    # xs = x * scale (group-broadcast)
                    nc.vector.tensor_tensor(
                        xg, xg,
                        scale[:, :, None].to_broadcast((P, NGX, G)),
                        Alu.mult)
                    # q = rint(xs) via magic add/sub
                    nc.vector.tensor_scalar(xt[:], xt[:], MAGIC, MAGIC,
                                            Alu.add, Alu.subtract)
                    # x_q = q * (absmax/127) -> fp16
                    xq16 = xq16_pool.tile([P, K2], f16, tag="xq16",
                                          name=f"xq16_{mb}_{h}")
                    nc.vector.tensor_tensor(
                        xq16.rearrange("p (g e) -> p g e", e=G), xg,
                        inv[:, :, None].to_broadcast((P, NGX, G)),
                        Alu.mult)
                    nc.sync.dma_start_transpose(
                        xq_t[:, h * KS2:(h + 1) * KS2,
                             mb * P:(mb + 1) * P], xq16[:])

            # -------- weight ternarize --------
            wt_tiles = {}
            bp_tiles = {}

            def wternarize(oc):
                whs = w_stage.pop(oc)
                gs = small.tile([P, NGW], f32, tag="gs")
                for h in range(2):
                    nc.vector.tensor_reduce(
                        gs[:, h * NGX:(h + 1) * NGX],
                        whs[h].rearrange("p (g e) -> p g e", e=G),
                        Ax.X, Alu.add, apply_absolute_value=True)
                # s = max(mean|row|, eps), two-stage compensated sum
                hql = small.tile([P, 2 * NGW], f32, tag="hql")
                nc.vector.tensor_scalar(hql[:, :NGW], gs[:], MAGIC2, MAGIC2,
                                        Alu.add, Alu.subtract)
                nc.vector.tensor_tensor(hql[:, NGW:], gs[:], hql[:, :NGW],
                                        Alu.subtract)
                shl = small.tile([P, 2], f32, tag="shl")
                nc.vector.tensor_reduce(
                    shl[:], hql.rearrange("p (a b) -> p a b", a=2),
                    Ax.X, Alu.add)
                ssum = small.tile([P, 1], f32, tag="ssum")
                nc.vector.tensor_tensor(ssum[:], shl[:, 0:1], shl[:, 1:2],
                                        Alu.add)
                sv = small.tile([P, 1], f32, tag="svv")
                nc.vector.tensor_scalar(sv[:], ssum[:],
                                        float(np.float32(1.0 / K)),
                                        EPS, Alu.mult, Alu.max)
                # eviction scale is 0.5*s (the sign-sum below is 2t)
                bp = sv_pool.tile([P, 1], f32, tag="bp", name=f"bp{oc}")
                nc.vector.tensor_scalar(bp[:], sv[:], 0.5, None, Alu.mult)
                bp_tiles[oc] = bp
                bn = small.tile([P, 1], f32, tag="bn")
                nc.vector.tensor_scalar(bn[:], sv[:], -0.5, None, Alu.mult)
                # 2t = sign(w-0.5s) + sign(w+0.5s); exact comparisons.
                # Boundary |w|==0.5s gives +-1; the f32 grid-of-2 magic
                # round maps it to 0 (= round-half-even of w/s).
                wt = wt_pool.tile([P, KSUB, P], f16, tag="wt",
                                  name=f"wt{oc}")
                wt_tiles[oc] = wt
                for h in range(2):
                    sga = sg_pool.tile([P, K2], f16, tag="sga",
                                       name=f"sga{oc}_{h}")
                    nc.scalar.activation(out=sga[:], in_=whs[h][:],
                                         func=Act.Sign, bias=bn[:],
                                         scale=1.0)
                    sgb = sg_pool.tile([P, K2], f16, tag="sgb",
                                       name=f"sgb{oc}_{h}")
                    nc.scalar.activation(out=sgb[:], in_=whs[h][:],
                                         func=Act.Sign, bias=bp[:],
                                         scale=1.0)
                    nc.vector.tensor_tensor(sga[:], sga[:], sgb[:], Alu.add)
                    nc.vector.tensor_scalar(sga[:], sga[:], FIXC, FIXC,
                                            Alu.add, Alu.subtract)
                    nc.sync.dma_start_transpose(
                        wt[:, h * KS2:(h + 1) * KS2, :], sga[:])

            # -------- matmul + eviction --------
            # Ternarize runs one oc ahead of the matmuls and evictions run
            # one oc behind, so the Act queue never head-of-line blocks the
            # next oc's Sign passes behind an eviction that waits on PE.
            ps_tiles = {}

            def evict(oc):
                bp = bp_tiles.pop(oc)
                for mh in range(MH):
                    ps = ps_tiles.pop((oc, mh))
                    ev = ev_pool.tile([P, 512], f32)
                    nc.scalar.activation(out=ev[:], in_=ps[:],
                                         func=Act.Copy, scale=bp[:])
                    nc.gpsimd.dma_start(
                        oap[oc * P:(oc + 1) * P, mh * 512:(mh + 1) * 512],
                        ev[:])

            wternarize(0)
            if OC > 1:
                wternarize(1)
            for oc in range(OC):
                if oc + 3 < OC:
                    wload(oc + 3)
                if oc + 2 < OC:
                    wternarize(oc + 2)
                wt = wt_tiles.pop(oc)
                for mh in range(MH):
                    ps = ps_pool.tile([P, 512], f32)
                    ps_tiles[(oc, mh)] = ps
                    for ks in range(KSUB):
                        nc.tensor.matmul(
                            ps[:], wt[:, ks, :],
                            xq_t[:, ks, mh * 512:(mh + 1) * 512],
                            start=(ks == 0), stop=(ks == KSUB - 1))
                if oc >= 1:
                    evict(oc - 1)
            evict(OC - 1)

    nc.compile()
    return nc


def _get_nc():
    if "nc" not in _cache:
        _cache["nc"] = _build(M_C, D_IN, D_OUT)
    return _cache["nc"]


def run(x, weight, trace=False):
    """Run on 8 NeuronCores; returns (full output [B,S,D_OUT], results obj)."""
    from concourse.bass_utils import run_bass_kernel_spmd

    x = np.ascontiguousarray(np.asarray(x, dtype=np.float32))
    w = np.ascontiguousarray(np.asarray(weight, dtype=np.float32))
    assert x.shape == (B, S, D_IN) and w.shape == (D_OUT, D_IN)
    xf = x.reshape(M_TOT, D_IN)
    nc = _get_nc()
    in_maps = [
        {"x": np.ascontiguousarray(xf[c * M_C:(c + 1) * M_C]), "w": w}
        for c in range(N_CORES)
    ]
    res = run_bass_kernel_spmd(nc, in_maps, core_ids=list(range(N_CORES)),
                               trace=trace)
    outf = np.concatenate(
        [res.results[c]["out"].T for c in range(N_CORES)], axis=0)
    return np.ascontiguousarray(outf).reshape(B, S, D_OUT), res


def kernel(x, weight):
    out, _ = run(x, weight)
    return out


# revision 25
# speedup vs baseline: 1.0621x; 1.0271x over previous
"""BitLinear 1.58 (nn_BitLinear158) Trainium2 Bass kernel.

Problem: x:[4,2048,4096] f32, weight:[4096,4096] f32 ->
         absmax-group-quantized x (8-bit fake quant, groups of 64) @
         ternary-quantized weight.T (per-row absmean scale) -> [4,2048,4096].

Sharding: data-parallel over tokens (1024 tokens/core, full weight
replicated) — minimizes replicated elementwise work.

Per-core kernel (M=1024, K=4096, O=4096), engine-balanced so the tensor
engine (437us roofline at 2.4GHz) paces the pipeline:
  - PE: stationary = ternary weights [k,128o], moving = x_q [k,512m],
    psum [o,m]; 2048 matmuls stream at ~216ns each (full clock,
    ldweights pipelined).
  - DVE: the group reduces (x absmax, w abs-sum; reduces are DVE-only),
    the small scale chain, and the x round. ~300us.
  - Act (scalar): w loads (HWDGE), the two Sign passes per w half-tile
    (ternarize via sign(w-s/2)+sign(w+s/2), boundary fixed by a
    grid-of-2 magic round), psum eviction via Copy activation with
    scale=0.5*s as a per-partition AP (output orientation [o,m] makes
    the row scale per-partition). ~230us.
  - Pool (gpsimd): x scale-mult + dequant passes, w sign-sum combine +
    magic fix (USE_POOL), plus the SWDGE output stores. ~270us.
  - SP (sync): x loads + all xbar transposes (keeping transpose issue
    off the Act queue, which would head-of-line block behind evictions).
  - s computed with a two-stage compensated reduction (exact 2^-12-grid
    split) to track the f32 reference mean closely; ternary decisions
    are exact comparisons against +-0.5*s so there is no divide rounding.
"""
import sys

sys.path.insert(0, "/opt/trn_rl_repo")

import numpy as np

B, S, D_IN, D_OUT = 4, 2048, 4096, 4096
N_CORES = 8
M_TOT = B * S
M_C = M_TOT // N_CORES

P = 128
G = 64
MAGIC = float(1.5 * 2.0 ** 23)   # fp32 round-to-nearest-even trick
MAGIC2 = float(1.5 * 2.0 ** 11)  # quantize-to-2^-12-grid trick
FIXC = float(1.5 * 2.0 ** 24)    # f32 grid-of-2 round (ties-to-even)
EPS = 1e-5
QMAX = 127.0
INV_QMAX = float(np.float32(1.0 / 127.0))

# Note: gpsimd/Pool compute was measured and rejected — Pool TENSOR_SCALAR
# runs ~29us/[128,2048] (software DSP path) and even Pool TENSOR_TENSOR
# (~3.6us) degrades concurrent DVE throughput ~2x via SBUF port contention.

_cache = {}


def _build(M, K, O):
    import concourse.bass as bass
    import concourse.tile as tile
    from concourse import bacc, mybir

    f32 = mybir.dt.float32
    f16 = mybir.dt.float16
    Alu = mybir.AluOpType
    Act = mybir.ActivationFunctionType
    Ax = mybir.AxisListType

    K2 = K // 2          # 2048, half-row staged per DMA
    KSUB = K // P        # 32 contraction chunks
    KS2 = KSUB // 2      # 16 chunks per half
    MB = M // P          # token blocks
    OC = O // P          # out-feature blocks
    MH = M // 512        # psum column halves
    NGX = K2 // G        # 32 quant groups per x half-tile
    NGW = K // G         # 64 abs-mean groups per w row

    nc = bacc.Bacc("TRN2", target_bir_lowering=False, num_devices=1)
    x = nc.dram_tensor("x", [M, K], f32, kind="ExternalInput")
    w = nc.dram_tensor("w", [O, K], f32, kind="ExternalInput")
    # transposed output [O, M]; host untransposes at gather time
    out = nc.dram_tensor("out", [O, M], f32, kind="ExternalOutput")

    xap, wap, oap = x.ap(), w.ap(), out.ap()

    with tile.TileContext(nc) as tc:
        with (
            tc.tile_pool(name="xq", bufs=1) as xq_pool,
            tc.tile_pool(name="xst", bufs=3) as xst,
            tc.tile_pool(name="xq16", bufs=2) as xq16_pool,
            tc.tile_pool(name="wst", bufs=4) as wst,
            tc.tile_pool(name="sg", bufs=4) as sg_pool,
            tc.tile_pool(name="wt", bufs=3) as wt_pool,
            tc.tile_pool(name="small", bufs=3) as small,
            tc.tile_pool(name="sv", bufs=8) as sv_pool,
            tc.tile_pool(name="ev", bufs=2) as ev_pool,
            tc.tile_pool(name="ps", bufs=6, space="PSUM") as ps_pool,
        ):
            # -------- activation loads (SP queue, issued first) --------
            x_stage = {}
            for mb in range(MB):
                for h in range(2):
                    xt = xst.tile([P, K2], f32, tag="xst",
                                  name=f"xt{mb}_{h}")
                    nc.sync.dma_start(
                        xt[:], xap[mb * P:(mb + 1) * P, h * K2:(h + 1) * K2])
                    x_stage[(mb, h)] = xt

            # -------- weight loads (SP queue, lookahead) --------
            w_stage = {}

            def wload(oc):
                tiles = []
                for h in range(2):
                    wh = wst.tile([P, K2], f32, tag="wst",
                                  name=f"wh{oc}_{h}")
                    nc.sync.dma_start(
                        wh[:], wap[oc * P:(oc + 1) * P, h * K2:(h + 1) * K2])
                    tiles.append(wh)
                w_stage[oc] = tiles

            for _oc in range(min(3, OC)):
                wload(_oc)

            # -------- activation quantization + transpose --------
            xq_t = xq_pool.tile([P, KSUB, M], f16, name="xq_t")
            for mb in range(MB):
                for h in range(2):
                    xt = x_stage.pop((mb, h))
                    xg = xt.rearrange("p (g e) -> p g e", e=G)
                    am = small.tile([P, NGX], f32, tag="am")
                    nc.vector.tensor_reduce(am[:], xg, Ax.X, Alu.max,
                                            apply_absolute_value=True)
                    am2 = small.tile([P, NGX], f32, tag="am2")
                    nc.vector.tensor_scalar(am2[:], am[:], EPS, None, Alu.max)
                    rc = small.tile([P, NGX], f32, tag="rc")
                    nc.vector.reciprocal(rc[:], am2[:])
                    scale = small.tile([P, NGX], f32, tag="scale")
                    nc.vector.tensor_scalar(scale[:], rc[:], QMAX, None,
                                            Alu.mult)
                    inv = small.tile([P, NGX], f32, tag="inv")
                    nc.vector.tensor_scalar(inv[:], am2[:], INV_QMAX, None,
                                            Alu.mult)
                    # xs = x * scale (group-broadcast)
                    nc.vector.tensor_tensor(
                        xg, xg,
                        scale[:, :, None].to_broadcast((P, NGX, G)),
                        Alu.mult)
                    # q = rint(xs) via magic add/sub
                    nc.vector.tensor_scalar(xt[:], xt[:], MAGIC, MAGIC,
                                            Alu.add, Alu.subtract)
                    # x_q = q * (absmax/127) -> fp16
                    xq16 = xq16_pool.tile([P, K2], f16, tag="xq16",
                                          name=f"xq16_{mb}_{h}")
                    nc.vector.tensor_tensor(
                        xq16.rearrange("p (g e) -> p g e", e=G), xg,
                        inv[:, :, None].to_broadcast((P, NGX, G)),
                        Alu.mult)
                    nc.sync.dma_start_transpose(
                        xq_t[:, h * KS2:(h + 1) * KS2,
                             mb * P:(mb + 1) * P], xq16[:])

            # -------- weight ternarize --------
            # Software-pipelined in two stages so the DVE queue never
            # head-of-line blocks: stage A (reduces + scale chain + Act Sign
            # issues) runs two ocs ahead; stage B (sign-sum + boundary fix +
            # transpose) runs one oc ahead, by which time the Signs are done.
            wt_tiles = {}
            bp_tiles = {}
            sg_tiles = {}

            def wternA(oc):
                whs = w_stage.pop(oc)
                gs = small.tile([P, NGW], f32, tag="gs")
                for h in range(2):
                    nc.vector.tensor_reduce(
                        gs[:, h * NGX:(h + 1) * NGX],
                        whs[h].rearrange("p (g e) -> p g e", e=G),
                        Ax.X, Alu.add, apply_absolute_value=True)
                # s = max(mean|row|, eps), two-stage compensated sum
                hql = small.tile([P, 2 * NGW], f32, tag="hql")
                nc.vector.tensor_scalar(hql[:, :NGW], gs[:], MAGIC2, MAGIC2,
                                        Alu.add, Alu.subtract)
                nc.vector.tensor_tensor(hql[:, NGW:], gs[:], hql[:, :NGW],
                                        Alu.subtract)
                shl = small.tile([P, 2], f32, tag="shl")
                nc.vector.tensor_reduce(
                    shl[:], hql.rearrange("p (a b) -> p a b", a=2),
                    Ax.X, Alu.add)
                ssum = small.tile([P, 1], f32, tag="ssum")
                nc.vector.tensor_tensor(ssum[:], shl[:, 0:1], shl[:, 1:2],
                                        Alu.add)
                sv = small.tile([P, 1], f32, tag="svv")
                nc.vector.tensor_scalar(sv[:], ssum[:],
                                        float(np.float32(1.0 / K)),
                                        EPS, Alu.mult, Alu.max)
                # eviction scale is 0.5*s (the sign-sum below is 2t)
                bp = sv_pool.tile([P, 1], f32, tag="bp", name=f"bp{oc}")
                nc.vector.tensor_scalar(bp[:], sv[:], 0.5, None, Alu.mult)
                bp_tiles[oc] = bp
                bn = small.tile([P, 1], f32, tag="bn")
                nc.vector.tensor_scalar(bn[:], sv[:], -0.5, None, Alu.mult)
                # 2t = sign(w-0.5s) + sign(w+0.5s); exact comparisons.
                # Boundary |w|==0.5s gives +-1; the f32 grid-of-2 magic
                # round maps it to 0 (= round-half-even of w/s).
                tiles = []
                for h in range(2):
                    sga = sg_pool.tile([P, K2], f16, tag="sga",
                                       name=f"sga{oc}_{h}")
                    nc.scalar.activation(out=sga[:], in_=whs[h][:],
                                         func=Act.Sign, bias=bn[:],
                                         scale=1.0)
                    sgb = sg_pool.tile([P, K2], f16, tag="sgb",
                                       name=f"sgb{oc}_{h}")
                    nc.scalar.activation(out=sgb[:], in_=whs[h][:],
                                         func=Act.Sign, bias=bp[:],
                                         scale=1.0)
                    tiles.append((sga, sgb))
                sg_tiles[oc] = tiles

            def wternB(oc):
                tiles = sg_tiles.pop(oc)
                wt = wt_pool.tile([P, KSUB, P], f16, tag="wt",
                                  name=f"wt{oc}")
                wt_tiles[oc] = wt
                for h in range(2):
                    sga, sgb = tiles[h]
                    nc.vector.tensor_tensor(sga[:], sga[:], sgb[:], Alu.add)
                    nc.vector.tensor_scalar(sga[:], sga[:], FIXC, FIXC,
                                            Alu.add, Alu.subtract)
                    nc.sync.dma_start_transpose(
                        wt[:, h * KS2:(h + 1) * KS2, :], sga[:])

            # -------- matmul + eviction --------
            # Ternarize runs one oc ahead of the matmuls and evictions run
            # one oc behind, so the Act queue never head-of-line blocks the
            # next oc's Sign passes behind an eviction that waits on PE.
            ps_tiles = {}

            def evict(oc):
                bp = bp_tiles.pop(oc)
                for mh in range(MH):
                    ps = ps_tiles.pop((oc, mh))
                    ev = ev_pool.tile([P, 512], f32)
                    nc.scalar.activation(out=ev[:], in_=ps[:],
                                         func=Act.Copy, scale=bp[:])
                    nc.gpsimd.dma_start(
                        oap[oc * P:(oc + 1) * P, mh * 512:(mh + 1) * 512],
                        ev[:])

            wternA(0)
            wternB(0)
            if OC > 1:
                wternA(1)
            for oc in range(OC):
                if oc + 3 < OC:
                    wload(oc + 3)
                if oc + 2 < OC:
                    wternA(oc + 2)
                if oc + 1 < OC:
                    wternB(oc + 1)
                wt = wt_tiles.pop(oc)
                for mh in range(MH):
                    ps = ps_pool.tile([P, 512], f32)
                    ps_tiles[(oc, mh)] = ps
                    for ks in range(KSUB):
                        nc.tensor.matmul(
                            ps[:], wt[:, ks, :],
                            xq_t[:, ks, mh * 512:(mh + 1) * 512],
                            start=(ks == 0), stop=(ks == KSUB - 1))
                if oc >= 1:
                    evict(oc - 1)
            evict(OC - 1)

    nc.compile()
    return nc


def _get_nc():
    if "nc" not in _cache:
        _cache["nc"] = _build(M_C, D_IN, D_OUT)
    return _cache["nc"]


def run(x, weight, trace=False):
    """Run on 8 NeuronCores; returns (full output [B,S,D_OUT], results obj)."""
    from concourse.bass_utils import run_bass_kernel_spmd

    x = np.ascontiguousarray(np.asarray(x, dtype=np.float32))
    w = np.ascontiguousarray(np.asarray(weight, dtype=np.float32))
    assert x.shape == (B, S, D_IN) and w.shape == (D_OUT, D_IN)
    xf = x.reshape(M_TOT, D_IN)
    nc = _get_nc()
    in_maps = [
        {"x": np.ascontiguousarray(xf[c * M_C:(c + 1) * M_C]), "w": w}
        for c in range(N_CORES)
    ]
    res = run_bass_kernel_spmd(nc, in_maps, core_ids=list(range(N_CORES)),
                               trace=trace)
    outf = np.concatenate(
        [res.results[c]["out"].T for c in range(N_CORES)], axis=0)
    return np.ascontiguousarray(outf).reshape(B, S, D_OUT), res


def kernel(x, weight):
    out, _ = run(x, weight)
    return out
